# revision 23
# baseline (speedup 1.0000x reference)
"""Multi-Head Latent Attention (MLA) TRN2 Bass kernel, 8-core parallel, fp16.

Sharding: batch x heads. Cores 0-3 own batch 0, cores 4-7 batch 1; within a
batch group each core owns 4 heads (tensor-parallel on q/kv_up/o_proj).
Each core computes the latent projection for its batch (4x replicated),
q/kv projections for its heads, attention, and a partial o_proj; the host
sums the 4 partials per batch and stacks the batches.

All data is fp16 (PE runs fp16 at 1 col/cycle like fp32r, but DVE gets 2x
throughput and DMA traffic halves; rel-err budget 2e-2 >> fp16's ~5e-4).

The kv_down/kv_up pair is folded on the host (Wk = Wdown @ Wup, computed
in fp64 and rounded to fp16 once): kv = x @ Wk costs the same PE time as
the q projection and removes the latent projection (and its 4x replication
across the batch group) from the device entirely.

Dataflow (everything transposed, no on-device transposes except kv_nat):
  xT [D, S] (host-side transpose, per batch) ->
  qT = Wq^T xT, kvT = Wk^T xT                               (all [*, S])
  kv_nat[kt] = PE-transpose(kvT[:, kt-block])               ([keys, Dh])
  scoresT[keys, q] = kvT^T(slice) qT;  expT = exp(scoresT * scale)
  outT[Dh, q]  = kv_nat^T(slice) expT  (accumulate over key tiles)
  denom[*, q]  = ones^T acc            (acc = fp16 DVE sums of expT tiles)
  outT_norm    = outT * (1/denom);  final = outT_norm^T Wo   ([S, D])
Softmax max-subtraction is skipped: scores are ~N(0, 0.037), |s| < ~1.5.

The attention inner loop is ACT(exp)-bound, so all PE work that is not
attention itself (kv projections for heads 1-3, q projections beyond head
0's first half, o_proj chunks) is cut into ~850ns chunks and drained one
per key-tile step inside the attention loops, keeping PE busy while ACT
crunches exp. x stays fully resident in SBUF so q projections can drain
late without re-reading DRAM. qT/kvT/kv_nat/outT all live in SBUF; the
only DRAM round trips are the inputs and the final fp16 partial output.
"""
import sys

sys.path.insert(0, "/opt/trn_rl_repo")

import numpy as np  # noqa: E402

B = 2
S = 2048
D = 2048
H = 16
DH = 128
DL = 512
P = 128
N_CORES = 8
H_LOC = 4                     # heads per core
HW = H_LOC * DH               # 512
SCALE = float(1.0 / np.sqrt(DH))

D_T = D // P                  # 16
L_T = DL // P                 # 4
A0_W = [256, 256, 512, 512, 512]      # xs DMA slice widths (sum = S)
A0_OFF = [0, 256, 512, 1024, 1536]
KT = S // P                   # 16 key tiles
QW = 512                      # query half-width in phase B
NT = D // QW                  # 4 o_proj col chunks


def _build_nc():
    import concourse.tile as tile
    import concourse.mybir as mybir
    from concourse import bacc

    f32 = mybir.dt.float32
    f16 = mybir.dt.float16
    EXP = mybir.ActivationFunctionType.Exp

    nc = bacc.Bacc("TRN2", target_bir_lowering=False, debug=False)

    xT = nc.dram_tensor("xT", [D, S], f16, kind="ExternalInput").ap()
    wqB = nc.dram_tensor("wqB", [HW, D], f16, kind="ExternalInput").ap()
    wkB = nc.dram_tensor("wkB", [HW, D], f16, kind="ExternalInput").ap()
    woB = nc.dram_tensor("woB", [HW, D], f16, kind="ExternalInput").ap()
    ones_d = nc.dram_tensor("ones", [P, P], f16, kind="ExternalInput").ap()
    eye_d = nc.dram_tensor("eye", [P, P], f16, kind="ExternalInput").ap()
    out_d = nc.dram_tensor("out", [S, D], f16, kind="ExternalOutput").ap()

    with tile.TileContext(nc) as tc:
        with tc.tile_pool(name="w", bufs=1) as wp, \
             tc.tile_pool(name="big", bufs=1) as bigp, \
             tc.tile_pool(name="sm", bufs=1) as smp, \
             tc.tile_pool(name="ps", bufs=1, space="PSUM") as psp:

            # ---- initial DMAs in need order (one hw queue; issue-bound) ----
            wq_t = wp.tile([P, H_LOC, D], f16, tag="wq", name="wq")
            ones_t = wp.tile([P, P], f16, tag="ones", name="ones")
            eye_t = wp.tile([P, P], f16, tag="eye", name="eye")
            warm = smp.tile([P, P], f16, tag="warm", name="warm")

            xs = wp.tile([P, D_T, S], f16, tag="xs", name="xs")

            def xs_dma_g(j, qtr):
                j0, W = A0_OFF[j], A0_W[j]
                dsl = slice(qtr * 4, qtr * 4 + 4)
                nc.sync.dma_start(
                    xs[:, dsl, j0:j0 + W],
                    xT[qtr * 4 * P:(qtr * 4 + 4) * P, j0:j0 + W]
                    .rearrange("(t p) w -> p t w", p=P))

            def xs_dma(j):
                for qtr in range(4):
                    xs_dma_g(j, qtr)

            # wq0 first: q(0,0) seg0 is gated on it plus the first two xs
            # quarter-slices, so it must not queue behind 2 MB of x.
            # wk0/eye follow xs(1) so kv(0,j2=0)+t_seg(0) can fill the PE
            # hole while q(0,1) waits for xs(2).
            nc.sync.dma_start(wq_t[:, 0, :], wqB[0:P, :])
            # interleave the two slices' quarters: q(0,0) seg s is gated on
            # quarter s of BOTH slices, so pair them up
            for qtr in range(4):
                xs_dma_g(0, qtr)
                xs_dma_g(1, qtr)
            wk_t = wp.tile([P, H_LOC, D], f16, tag="wk", name="wk")
            nc.sync.dma_start(wk_t[:, 0, :], wkB[0:P, :])
            nc.sync.dma_start(eye_t[:], eye_d[:, :])
            xs_dma(2)
            nc.sync.dma_start(ones_t[:], ones_d[:, :])
            # warm the Exp activation table off the critical path
            nc.scalar.activation(warm[:], ones_t[:], EXP, scale=1.0)
            xs_dma(3)
            nc.sync.dma_start(wk_t[:, 1, :], wkB[P:2 * P, :])
            nc.sync.dma_start(wq_t[:, 1, :], wqB[P:2 * P, :])
            xs_dma(4)
            for m in range(2, H_LOC):
                nc.sync.dma_start(wq_t[:, m, :], wqB[m * P:(m + 1) * P, :])
                nc.sync.dma_start(wk_t[:, m, :], wkB[m * P:(m + 1) * P, :])
            wo_t = wp.tile([P, H_LOC, D], f16, tag="wo", name="wo")
            for h in range(H_LOC):
                nc.sync.dma_start(wo_t[:, h, :], woB[h * P:(h + 1) * P, :])

            qT = [bigp.tile([P, S], f16, tag=f"qt{m}", name=f"qt{m}")
                  for m in range(H_LOC)]
            kvT = [bigp.tile([P, S], f16, tag=f"kvt{h}", name=f"kvt{h}")
                   for h in range(H_LOC)]
            # kv_nat blocks: kvnB[:, b, q, h*128:(h+1)*128] = kv_nat[4b+q, h]
            kvnB = bigp.tile([P, 4, 4, HW], f16, tag="kvn", name="kvn")

            # ---- chunk makers (emitted inline or drained inside B) ----
            kv_hold = {}

            def q_chunks(h, j2):
                """q projection for head h, 512-col slice j2: 4 chunks of
                4 matmuls continuing one psum chain (lazy slot alloc)."""
                jsl = slice(j2 * 512, (j2 + 1) * 512)
                holder = []

                def seg(s, h=h, j2=j2, jsl=jsl, holder=holder):
                    if s == 0:
                        holder.append(psp.tile([P, QW], f32, tag="D", bufs=2,
                                               name=f"psQ_{h}_{j2}"))
                    ps = holder[0]
                    for dt_i in range(4 * s, 4 * s + 4):
                        nc.tensor.matmul(ps[:],
                                         wq_t[:, h, dt_i * P:(dt_i + 1) * P],
                                         xs[:, dt_i, jsl],
                                         start=(dt_i == 0),
                                         stop=(dt_i == D_T - 1))
                    if s == 3:
                        nc.scalar.copy(qT[h][:, jsl], ps[:])
                return [lambda s=s: seg(s) for s in range(4)]

            def kv_chunks(h):
                """Folded kv projection (x @ Wk) for head h: 4 slices of 4
                chunks, plus 4 kv_nat transpose chunks."""
                def kv_seg(j2, s, h=h):
                    jsl = slice(j2 * 512, (j2 + 1) * 512)
                    if s == 0:
                        kv_hold[(h, j2)] = psp.tile(
                            [P, QW], f32, tag="D", bufs=2, name=f"psK_{h}_{j2}")
                    ps = kv_hold[(h, j2)]
                    for dt_i in range(4 * s, 4 * s + 4):
                        nc.tensor.matmul(ps[:],
                                         wk_t[:, h, dt_i * P:(dt_i + 1) * P],
                                         xs[:, dt_i, jsl],
                                         start=(dt_i == 0),
                                         stop=(dt_i == D_T - 1))
                    if s == 3:
                        nc.vector.tensor_copy(kvT[h][:, jsl], ps[:])

                def t_seg(j2, h=h):
                    pt = psp.tile([P, QW], f16, tag="D", bufs=2,
                                  name=f"pt_{h}_{j2}")
                    for q in range(4):
                        kt = j2 * 4 + q
                        nc.tensor.transpose(pt[:, q * P:(q + 1) * P],
                                            kvT[h][:, kt * P:(kt + 1) * P],
                                            eye_t[:])
                    nc.vector.tensor_copy(
                        kvnB[:, j2, :, h * P:(h + 1) * P],
                        pt[:].rearrange("p (q c) -> p q c", c=P))

                return ([(lambda j2=j2, s=s: kv_seg(j2, s))
                         for j2 in range(4) for s in range(4)]
                        + [(lambda j2=j2: t_seg(j2)) for j2 in range(4)])

            outT = bigp.tile([P, H_LOC, S], f16, tag="outT", name="outT")

            def oproj_chunk(st, nt, idx):
                pc = psp.tile([P, QW], f32, tag="D", bufs=2,
                              name=f"pc_{st}_{nt}")
                for h in range(H_LOC):
                    nc.tensor.matmul(pc[:],
                                     outT[:, h, st * P:(st + 1) * P],
                                     wo_t[:, h, nt * QW:(nt + 1) * QW],
                                     start=(h == 0), stop=(h == H_LOC - 1))
                fin = smp.tile([P, QW], f16, tag="fin", bufs=4,
                               name=f"fin_{st}_{nt}")
                if idx % 2 == 0:
                    nc.vector.tensor_copy(fin[:], pc[:])
                else:
                    nc.scalar.copy(fin[:], pc[:])
                osl = out_d[st * P:(st + 1) * P, nt * QW:(nt + 1) * QW]
                if st == 15:
                    # split the very last chunks across both queues: a
                    # single queue flushes 64 KB at ~20 GB/s and would
                    # stall the teardown drain by ~3 us
                    nc.gpsimd.dma_start(osl[:, 0:QW // 2], fin[:, 0:QW // 2])
                    nc.sync.dma_start(osl[:, QW // 2:QW], fin[:, QW // 2:QW])
                else:
                    eng = nc.gpsimd if idx % 2 == 0 else nc.sync
                    eng.dma_start(osl, fin[:])

            # ---- inline: q(h0) first half and kv/kvn for h0, ordered by
            # xs slice arrival: kv(0,j2=0)+transpose fill the PE hole
            # between q(0,0) (needs xs0/xs1) and q(0,1) (needs xs2) ----
            kvc0 = kv_chunks(0)
            for ch in q_chunks(0, 0):
                ch()
            for s in range(4):
                kvc0[s]()
            kvc0[16]()
            for ch in q_chunks(0, 1):
                ch()
            for j2 in range(1, 4):
                for s in range(4):
                    kvc0[j2 * 4 + s]()
                kvc0[16 + j2]()

            # ---- drain deck for the attention loops ----
            deck = []
            deck += kv_chunks(1) + q_chunks(1, 0) + q_chunks(1, 1)
            deck += kv_chunks(2) + q_chunks(2, 0) + q_chunks(2, 1)
            deck += kv_chunks(3) + q_chunks(3, 0) + q_chunks(3, 1)
            deck += q_chunks(0, 2) + q_chunks(0, 3)
            deck += q_chunks(1, 2) + q_chunks(1, 3)
            deck += q_chunks(2, 2) + q_chunks(2, 3)
            deck += q_chunks(3, 2) + q_chunks(3, 3)

            # ---- Phase B: attention, query halves of 1024 ----
            for qp in range(2):
                for hh in range(H_LOC):
                    qsl0 = qp * 1024
                    ps_o = [psp.tile([P, QW], f32, tag=f"O{i}", bufs=1,
                                     name=f"pso_{qp}_{hh}_{i}")
                            for i in range(2)]
                    acc_d = smp.tile([P, 1024], f16, tag="accd", bufs=2,
                                     name=f"accd_{qp}_{hh}")
                    acc_g = smp.tile([P, 1024], f16, tag="accg", bufs=2,
                                     name=f"accg_{qp}_{hh}")
                    es = [None] * KT

                    def _consume(kt, ps_o=ps_o, acc_d=acc_d, acc_g=acc_g,
                                 es=es, hh=hh):
                        e = es[kt]
                        for i in range(2):
                            nc.tensor.matmul(ps_o[i][:],
                                             kvnB[:, kt // 4, kt % 4,
                                                  hh * P:(hh + 1) * P],
                                             e[:, i * QW:(i + 1) * QW],
                                             start=(kt == 0),
                                             stop=(kt == KT - 1))
                        acc = acc_d if kt % 2 == 0 else acc_g
                        if kt < 2:
                            nc.vector.tensor_copy(acc[:], e[:])
                        else:
                            nc.vector.tensor_add(acc[:], acc[:], e[:])

                    for kt in range(KT):
                        ps_s = psp.tile([P, 1024], f32, tag="A", bufs=2,
                                        name=f"pss_{qp}_{hh}_{kt}")
                        for i in range(2):
                            nc.tensor.matmul(
                                ps_s[:, i * QW:(i + 1) * QW],
                                kvT[hh][:, kt * P:(kt + 1) * P],
                                qT[hh][:, qsl0 + i * QW:qsl0 + (i + 1) * QW],
                                start=True, stop=True)
                        e = smp.tile([P, 1024], f16, tag="e", bufs=4,
                                     name=f"e_{qp}_{hh}_{kt}")
                        nc.scalar.activation(e[:], ps_s[:], EXP, scale=SCALE)
                        es[kt] = e
                        if kt >= 1:
                            _consume(kt - 1)
                        if qp == 0:
                            # 2 pops while the deck is rich, 1 when it is
                            # nearly dry so late qp0 keeps PE filler too
                            for _ in range(2 if len(deck) > 24 else 1):
                                if deck:
                                    deck.pop(0)()
                        elif kt % 2 == 0 and deck:
                            deck.pop(0)()
                    _consume(KT - 1)
                    # denominators + fused evacuation: normalize ps_o on its
                    # way out of PSUM (one DVE pass instead of copy + mul).
                    # DVE pre-adds the two fp16 partial-sum tiles so PE
                    # contracts once; drain pops cover the DVE latency.
                    acc_s = smp.tile([P, 1024], f16, tag="accs", bufs=2,
                                     name=f"accs_{qp}_{hh}")
                    nc.vector.tensor_add(acc_s[:], acc_d[:], acc_g[:])
                    for i in range(2):
                        ps_d = psp.tile([P, QW], f32, tag="D", bufs=2,
                                        name=f"psd_{qp}_{hh}_{i}")
                        nc.tensor.matmul(ps_d[:], ones_t[:],
                                         acc_s[:, i * QW:(i + 1) * QW],
                                         start=True, stop=True)
                        if i == 0 and qp == 0 and deck:
                            deck.pop(0)()
                        rcp = smp.tile([P, QW], f32, tag="rcp", bufs=2,
                                       name=f"rcp_{qp}_{hh}_{i}")
                        nc.vector.reciprocal_approx_fast(out=rcp[:],
                                                         in_=ps_d[:])
                        osl = outT[:, hh, qsl0 + i * QW:qsl0 + (i + 1) * QW]
                        nc.vector.tensor_mul(osl, ps_o[i][:], rcp[:])
                    for _ in range(2 if qp == 0 else 0):
                        if deck:
                            deck.pop(0)()

                # queue o_proj chunks for this query half
                for st in range(qp * 8, (qp + 1) * 8):
                    for nt in range(NT):
                        deck.append(
                            lambda st=st, nt=nt, idx=st * NT + nt:
                            oproj_chunk(st, nt, idx))

            # drain remaining chunks
            for ch in deck:
                ch()

    nc.compile()
    return nc


_NC_CACHE = None


def _get_nc():
    global _NC_CACHE
    if _NC_CACHE is None:
        _NC_CACHE = _build_nc()
    return _NC_CACHE


def _run(x, W_q, W_kv_down, W_kv_up, W_o, trace=False):
    from concourse.bass_utils import run_bass_kernel_spmd

    x = np.asarray(x, dtype=np.float32)
    W_q = np.asarray(W_q, dtype=np.float32)
    W_kv_down = np.asarray(W_kv_down, dtype=np.float32)
    W_kv_up = np.asarray(W_kv_up, dtype=np.float32)
    W_o = np.asarray(W_o, dtype=np.float32)

    nc = _get_nc()

    f16 = np.float16
    # fold the latent pair: kv = x @ (Wdown @ Wup); product in fp64,
    # rounded to fp16 once
    Wk = (W_kv_down.astype(np.float64) @ W_kv_up.astype(np.float64))
    ones = np.ones((P, P), f16)
    eye = np.eye(P, dtype=f16)
    xT_b = [np.ascontiguousarray(x[b].T).astype(f16) for b in range(B)]

    in_maps = []
    for c in range(N_CORES):
        bc = c // 4
        hs = slice((c % 4) * HW, (c % 4 + 1) * HW)
        wq_l = W_q[:, hs]                    # [D, 512]
        wk_l = Wk[:, hs]                     # [D, 512]
        wo_l = W_o[hs, :]                    # [512, D]
        wqB = np.ascontiguousarray(
            wq_l.reshape(D_T, P, H_LOC, P).transpose(2, 1, 0, 3)
            .reshape(HW, D)).astype(f16)
        wkB = np.ascontiguousarray(
            wk_l.reshape(D_T, P, H_LOC, P).transpose(2, 1, 0, 3)
            .reshape(HW, D)).astype(f16)
        in_maps.append({
            "xT": xT_b[bc],
            "wqB": wqB,
            "wkB": wkB,
            "woB": np.ascontiguousarray(wo_l).astype(f16),
            "ones": ones,
            "eye": eye,
        })

    r = run_bass_kernel_spmd(nc, in_maps, list(range(N_CORES)), trace=trace)
    outs = []
    for bc in range(B):
        acc = None
        for i in range(4):
            part = r.results[4 * bc + i]["out"].astype(np.float64)
            acc = part if acc is None else acc + part
        outs.append(acc)
    return np.stack(outs).astype(np.float32), r


def kernel(x, W_q, W_kv_down, W_kv_up, W_o):
    out, _ = _run(x, W_q, W_kv_down, W_kv_up, W_o, trace=False)
    return out



# revision 24
# speedup vs baseline: 1.1875x; 1.1875x over previous
"""Multi-Head Latent Attention (MLA) TRN2 Bass kernel, 8-core parallel, fp16.

Sharding: batch x heads. Cores 0-3 own batch 0, cores 4-7 batch 1; within a
batch group each core owns 4 heads (tensor-parallel on q/kv_up/o_proj).
Each core computes the latent projection for its batch (4x replicated),
q/kv projections for its heads, attention, and a partial o_proj; the host
sums the 4 partials per batch and stacks the batches.

All data is fp16 (PE runs fp16 at 1 col/cycle like fp32r, but DVE gets 2x
throughput and DMA traffic halves; rel-err budget 2e-2 >> fp16's ~5e-4).

The kv_down/kv_up pair is folded on the host (Wk = Wdown @ Wup, computed
in fp64 and rounded to fp16 once): kv = x @ Wk costs the same PE time as
the q projection and removes the latent projection (and its 4x replication
across the batch group) from the device entirely.

Dataflow (everything transposed, no on-device transposes except kv_nat):
  xT [D, S] (host-side transpose, per batch) ->
  qT = Wq^T xT, kvT = Wk^T xT                               (all [*, S])
  kv_nat[kt] = PE-transpose(kvT[:, kt-block])               ([keys, Dh])
  scoresT[keys, q] = kvT^T(slice) qT;  expT = exp(scoresT * scale)
  outT[Dh, q]  = kv_nat^T(slice) expT  (accumulate over key tiles)
  denom[*, q]  = ones^T acc            (acc = fp16 DVE sums of expT tiles)
  outT_norm    = outT * (1/denom);  final = outT_norm^T Wo   ([S, D])
Softmax max-subtraction is skipped: scores are ~N(0, 0.037), |s| < ~1.5.

The attention inner loop is ACT(exp)-bound, so all PE work that is not
attention itself (kv projections for heads 1-3, q projections beyond head
0's first half, o_proj chunks) is cut into ~850ns chunks and drained one
per key-tile step inside the attention loops, keeping PE busy while ACT
crunches exp. x stays fully resident in SBUF so q projections can drain
late without re-reading DRAM. qT/kvT/kv_nat/outT all live in SBUF; the
only DRAM round trips are the inputs and the final fp16 partial output.
"""
import sys

sys.path.insert(0, "/opt/trn_rl_repo")

import numpy as np  # noqa: E402

B = 2
S = 2048
D = 2048
H = 16
DH = 128
DL = 512
P = 128
N_CORES = 8
H_LOC = 4                     # heads per core
HW = H_LOC * DH               # 512
SCALE = float(1.0 / np.sqrt(DH))

D_T = D // P                  # 16
L_T = DL // P                 # 4
A0_W = [256, 256, 512, 512, 512]      # xs DMA slice widths (sum = S)
A0_OFF = [0, 256, 512, 1024, 1536]
KT = S // P                   # 16 key tiles
QW = 512                      # query half-width in phase B
NT = D // QW                  # 4 o_proj col chunks


def _build_nc():
    import concourse.tile as tile
    import concourse.mybir as mybir
    from concourse import bacc

    f32 = mybir.dt.float32
    f16 = mybir.dt.float16
    EXP = mybir.ActivationFunctionType.Exp

    nc = bacc.Bacc("TRN2", target_bir_lowering=False, debug=False)

    xT = nc.dram_tensor("xT", [D, S], f16, kind="ExternalInput").ap()
    wqB = nc.dram_tensor("wqB", [HW, D], f16, kind="ExternalInput").ap()
    wkB = nc.dram_tensor("wkB", [HW, D], f16, kind="ExternalInput").ap()
    woB = nc.dram_tensor("woB", [HW, D], f16, kind="ExternalInput").ap()
    ones_d = nc.dram_tensor("ones", [P, P], f16, kind="ExternalInput").ap()
    eye_d = nc.dram_tensor("eye", [P, P], f16, kind="ExternalInput").ap()
    out_d = nc.dram_tensor("out", [S, D], f16, kind="ExternalOutput").ap()

    with tile.TileContext(nc) as tc:
        with tc.tile_pool(name="w", bufs=1) as wp, \
             tc.tile_pool(name="big", bufs=1) as bigp, \
             tc.tile_pool(name="sm", bufs=1) as smp, \
             tc.tile_pool(name="ps", bufs=1, space="PSUM") as psp:

            # ---- initial DMAs in need order (one hw queue; issue-bound) ----
            wq_t = wp.tile([P, H_LOC, D], f16, tag="wq", name="wq")
            ones_t = wp.tile([P, P], f16, tag="ones", name="ones")
            eye_t = wp.tile([P, P], f16, tag="eye", name="eye")
            warm = smp.tile([P, P], f16, tag="warm", name="warm")

            xs = wp.tile([P, D_T, S], f16, tag="xs", name="xs")

            def xs_dma_g(j, qtr):
                j0, W = A0_OFF[j], A0_W[j]
                dsl = slice(qtr * 4, qtr * 4 + 4)
                nc.sync.dma_start(
                    xs[:, dsl, j0:j0 + W],
                    xT[qtr * 4 * P:(qtr * 4 + 4) * P, j0:j0 + W]
                    .rearrange("(t p) w -> p t w", p=P))

            def xs_dma(j):
                for qtr in range(4):
                    xs_dma_g(j, qtr)

            # wq0 first: q(0,0) seg0 is gated on it plus the first two xs
            # quarter-slices, so it must not queue behind 2 MB of x.
            # wk0/eye follow xs(1) so kv(0,j2=0)+t_seg(0) can fill the PE
            # hole while q(0,1) waits for xs(2).
            nc.sync.dma_start(wq_t[:, 0, :], wqB[0:P, :])
            xs_dma(0)
            xs_dma(1)
            wk_t = wp.tile([P, H_LOC, D], f16, tag="wk", name="wk")
            nc.sync.dma_start(wk_t[:, 0, :], wkB[0:P, :])
            nc.sync.dma_start(eye_t[:], eye_d[:, :])
            xs_dma(2)
            nc.sync.dma_start(ones_t[:], ones_d[:, :])
            # warm the Exp activation table off the critical path
            nc.scalar.activation(warm[:], ones_t[:], EXP, scale=1.0)
            xs_dma(3)
            nc.sync.dma_start(wk_t[:, 1, :], wkB[P:2 * P, :])
            nc.sync.dma_start(wq_t[:, 1, :], wqB[P:2 * P, :])
            xs_dma(4)
            for m in range(2, H_LOC):
                nc.sync.dma_start(wq_t[:, m, :], wqB[m * P:(m + 1) * P, :])
                nc.sync.dma_start(wk_t[:, m, :], wkB[m * P:(m + 1) * P, :])
            wo_t = wp.tile([P, H_LOC, D], f16, tag="wo", name="wo")
            for h in range(H_LOC):
                nc.sync.dma_start(wo_t[:, h, :], woB[h * P:(h + 1) * P, :])

            qT = [bigp.tile([P, S], f16, tag=f"qt{m}", name=f"qt{m}")
                  for m in range(H_LOC)]
            kvT = [bigp.tile([P, S], f16, tag=f"kvt{h}", name=f"kvt{h}")
                   for h in range(H_LOC)]
            # kv_nat blocks: kvnB[:, b, q, h*128:(h+1)*128] = kv_nat[4b+q, h]
            kvnB = bigp.tile([P, 4, 4, HW], f16, tag="kvn", name="kvn")

            # ---- chunk makers (emitted inline or drained inside B) ----
            kv_hold = {}

            def q_chunks(h, j2):
                """q projection for head h, 512-col slice j2: 4 chunks of
                4 matmuls continuing one psum chain (lazy slot alloc)."""
                jsl = slice(j2 * 512, (j2 + 1) * 512)
                holder = []

                def seg(s, h=h, j2=j2, jsl=jsl, holder=holder):
                    if s == 0:
                        holder.append(psp.tile([P, QW], f32, tag="D", bufs=2,
                                               name=f"psQ_{h}_{j2}"))
                    ps = holder[0]
                    for dt_i in range(4 * s, 4 * s + 4):
                        nc.tensor.matmul(ps[:],
                                         wq_t[:, h, dt_i * P:(dt_i + 1) * P],
                                         xs[:, dt_i, jsl],
                                         start=(dt_i == 0),
                                         stop=(dt_i == D_T - 1))
                    if s == 3:
                        nc.scalar.copy(qT[h][:, jsl], ps[:])
                return [lambda s=s: seg(s) for s in range(4)]

            def kv_chunks(h):
                """Folded kv projection (x @ Wk) for head h: 4 slices of 4
                chunks, plus 4 kv_nat transpose chunks."""
                def kv_seg(j2, s, h=h):
                    jsl = slice(j2 * 512, (j2 + 1) * 512)
                    if s == 0:
                        kv_hold[(h, j2)] = psp.tile(
                            [P, QW], f32, tag="D", bufs=2, name=f"psK_{h}_{j2}")
                    ps = kv_hold[(h, j2)]
                    for dt_i in range(4 * s, 4 * s + 4):
                        nc.tensor.matmul(ps[:],
                                         wk_t[:, h, dt_i * P:(dt_i + 1) * P],
                                         xs[:, dt_i, jsl],
                                         start=(dt_i == 0),
                                         stop=(dt_i == D_T - 1))
                    if s == 3:
                        nc.vector.tensor_copy(kvT[h][:, jsl], ps[:])

                def t_seg(j2, h=h):
                    pt = psp.tile([P, QW], f16, tag="D", bufs=2,
                                  name=f"pt_{h}_{j2}")
                    for q in range(4):
                        kt = j2 * 4 + q
                        nc.tensor.transpose(pt[:, q * P:(q + 1) * P],
                                            kvT[h][:, kt * P:(kt + 1) * P],
                                            eye_t[:])
                    nc.vector.tensor_copy(
                        kvnB[:, j2, :, h * P:(h + 1) * P],
                        pt[:].rearrange("p (q c) -> p q c", c=P))

                return ([(lambda j2=j2, s=s: kv_seg(j2, s))
                         for j2 in range(4) for s in range(4)]
                        + [(lambda j2=j2: t_seg(j2)) for j2 in range(4)])

            outT = bigp.tile([P, H_LOC, S], f16, tag="outT", name="outT")

            def oproj_chunk(st, nt, idx):
                pc = psp.tile([P, QW], f32, tag="D", bufs=2,
                              name=f"pc_{st}_{nt}")
                for h in range(H_LOC):
                    nc.tensor.matmul(pc[:],
                                     outT[:, h, st * P:(st + 1) * P],
                                     wo_t[:, h, nt * QW:(nt + 1) * QW],
                                     start=(h == 0), stop=(h == H_LOC - 1))
                fin = smp.tile([P, QW], f16, tag="fin", bufs=4,
                               name=f"fin_{st}_{nt}")
                if idx % 2 == 0:
                    nc.vector.tensor_copy(fin[:], pc[:])
                else:
                    nc.scalar.copy(fin[:], pc[:])
                eng = nc.gpsimd if idx % 2 == 0 else nc.sync
                eng.dma_start(
                    out_d[st * P:(st + 1) * P, nt * QW:(nt + 1) * QW], fin[:])

            # ---- inline: q(h0) first half and kv/kvn for h0, ordered by
            # xs slice arrival: kv(0,j2=0)+transpose fill the PE hole
            # between q(0,0) (needs xs0/xs1) and q(0,1) (needs xs2) ----
            kvc0 = kv_chunks(0)
            for ch in q_chunks(0, 0):
                ch()
            for s in range(4):
                kvc0[s]()
            kvc0[16]()
            for ch in q_chunks(0, 1):
                ch()
            for j2 in range(1, 4):
                for s in range(4):
                    kvc0[j2 * 4 + s]()
                kvc0[16 + j2]()

            # ---- drain deck for the attention loops ----
            deck = []
            deck += kv_chunks(1) + q_chunks(1, 0) + q_chunks(1, 1)
            deck += kv_chunks(2) + q_chunks(2, 0) + q_chunks(2, 1)
            deck += kv_chunks(3) + q_chunks(3, 0) + q_chunks(3, 1)
            deck += q_chunks(0, 2) + q_chunks(0, 3)
            deck += q_chunks(1, 2) + q_chunks(1, 3)
            deck += q_chunks(2, 2) + q_chunks(2, 3)
            deck += q_chunks(3, 2) + q_chunks(3, 3)

            # ---- Phase B: attention, query halves of 1024 ----
            for qp in range(2):
                for hh in range(H_LOC):
                    qsl0 = qp * 1024
                    ps_o = [psp.tile([P, QW], f32, tag=f"O{i}", bufs=1,
                                     name=f"pso_{qp}_{hh}_{i}")
                            for i in range(2)]
                    acc_d = smp.tile([P, 1024], f16, tag="accd", bufs=2,
                                     name=f"accd_{qp}_{hh}")
                    acc_g = smp.tile([P, 1024], f16, tag="accg", bufs=2,
                                     name=f"accg_{qp}_{hh}")
                    es = [None] * KT

                    def _consume(kt, ps_o=ps_o, acc_d=acc_d, acc_g=acc_g,
                                 es=es, hh=hh):
                        e = es[kt]
                        for i in range(2):
                            nc.tensor.matmul(ps_o[i][:],
                                             kvnB[:, kt // 4, kt % 4,
                                                  hh * P:(hh + 1) * P],
                                             e[:, i * QW:(i + 1) * QW],
                                             start=(kt == 0),
                                             stop=(kt == KT - 1))
                        acc = acc_d if kt % 2 == 0 else acc_g
                        if kt < 2:
                            nc.vector.tensor_copy(acc[:], e[:])
                        else:
                            nc.vector.tensor_add(acc[:], acc[:], e[:])

                    for kt in range(KT):
                        ps_s = psp.tile([P, 1024], f32, tag="A", bufs=2,
                                        name=f"pss_{qp}_{hh}_{kt}")
                        for i in range(2):
                            nc.tensor.matmul(
                                ps_s[:, i * QW:(i + 1) * QW],
                                kvT[hh][:, kt * P:(kt + 1) * P],
                                qT[hh][:, qsl0 + i * QW:qsl0 + (i + 1) * QW],
                                start=True, stop=True)
                        e = smp.tile([P, 1024], f16, tag="e", bufs=4,
                                     name=f"e_{qp}_{hh}_{kt}")
                        nc.scalar.activation(e[:], ps_s[:], EXP, scale=SCALE)
                        es[kt] = e
                        if kt >= 1:
                            _consume(kt - 1)
                        if qp == 0:
                            # 2 pops while the deck is rich, 1 when it is
                            # nearly dry so late qp0 keeps PE filler too
                            for _ in range(2 if len(deck) > 24 else 1):
                                if deck:
                                    deck.pop(0)()
                        elif kt % 2 == 0 and deck:
                            deck.pop(0)()
                    _consume(KT - 1)
                    # denominators + fused evacuation: normalize ps_o on its
                    # way out of PSUM (one DVE pass instead of copy + mul).
                    # DVE pre-adds the two fp16 partial-sum tiles so PE
                    # contracts once; drain pops cover the DVE latency.
                    acc_s = smp.tile([P, 1024], f16, tag="accs", bufs=2,
                                     name=f"accs_{qp}_{hh}")
                    nc.vector.tensor_add(acc_s[:], acc_d[:], acc_g[:])
                    for i in range(2):
                        ps_d = psp.tile([P, QW], f32, tag="D", bufs=2,
                                        name=f"psd_{qp}_{hh}_{i}")
                        nc.tensor.matmul(ps_d[:], ones_t[:],
                                         acc_s[:, i * QW:(i + 1) * QW],
                                         start=True, stop=True)
                        if i == 0 and qp == 0 and deck:
                            deck.pop(0)()
                        rcp = smp.tile([P, QW], f32, tag="rcp", bufs=2,
                                       name=f"rcp_{qp}_{hh}_{i}")
                        nc.vector.reciprocal_approx_fast(out=rcp[:],
                                                         in_=ps_d[:])
                        osl = outT[:, hh, qsl0 + i * QW:qsl0 + (i + 1) * QW]
                        nc.vector.tensor_mul(osl, ps_o[i][:], rcp[:])
                    for _ in range(2 if qp == 0 else 0):
                        if deck:
                            deck.pop(0)()

                # queue o_proj chunks for this query half
                for st in range(qp * 8, (qp + 1) * 8):
                    for nt in range(NT):
                        deck.append(
                            lambda st=st, nt=nt, idx=st * NT + nt:
                            oproj_chunk(st, nt, idx))

            # drain remaining chunks
            for ch in deck:
                ch()

    nc.compile()
    return nc


_NC_CACHE = None


def _get_nc():
    global _NC_CACHE
    if _NC_CACHE is None:
        _NC_CACHE = _build_nc()
    return _NC_CACHE


def _run(x, W_q, W_kv_down, W_kv_up, W_o, trace=False):
    from concourse.bass_utils import run_bass_kernel_spmd

    x = np.asarray(x, dtype=np.float32)
    W_q = np.asarray(W_q, dtype=np.float32)
    W_kv_down = np.asarray(W_kv_down, dtype=np.float32)
    W_kv_up = np.asarray(W_kv_up, dtype=np.float32)
    W_o = np.asarray(W_o, dtype=np.float32)

    nc = _get_nc()

    f16 = np.float16
    # fold the latent pair: kv = x @ (Wdown @ Wup); product in fp64,
    # rounded to fp16 once
    Wk = (W_kv_down.astype(np.float64) @ W_kv_up.astype(np.float64))
    ones = np.ones((P, P), f16)
    eye = np.eye(P, dtype=f16)
    xT_b = [np.ascontiguousarray(x[b].T).astype(f16) for b in range(B)]

    in_maps = []
    for c in range(N_CORES):
        bc = c // 4
        hs = slice((c % 4) * HW, (c % 4 + 1) * HW)
        wq_l = W_q[:, hs]                    # [D, 512]
        wk_l = Wk[:, hs]                     # [D, 512]
        wo_l = W_o[hs, :]                    # [512, D]
        wqB = np.ascontiguousarray(
            wq_l.reshape(D_T, P, H_LOC, P).transpose(2, 1, 0, 3)
            .reshape(HW, D)).astype(f16)
        wkB = np.ascontiguousarray(
            wk_l.reshape(D_T, P, H_LOC, P).transpose(2, 1, 0, 3)
            .reshape(HW, D)).astype(f16)
        in_maps.append({
            "xT": xT_b[bc],
            "wqB": wqB,
            "wkB": wkB,
            "woB": np.ascontiguousarray(wo_l).astype(f16),
            "ones": ones,
            "eye": eye,
        })

    r = run_bass_kernel_spmd(nc, in_maps, list(range(N_CORES)), trace=trace)
    outs = []
    for bc in range(B):
        acc = None
        for i in range(4):
            part = r.results[4 * bc + i]["out"].astype(np.float64)
            acc = part if acc is None else acc + part
        outs.append(acc)
    return np.stack(outs).astype(np.float32), r


def kernel(x, W_q, W_kv_down, W_kv_up, W_o):
    out, _ = _run(x, W_q, W_kv_down, W_kv_up, W_o, trace=False)
    return out

